# revision 2
# baseline (speedup 1.0000x reference)
"""MFA e-step (mixture of factor analyzers) on 8 Trainium2 NeuronCores.

Math: the reference computes per-component Gaussian log-likelihoods with
covariance C_k = Lam_k Lam_k^T + diag(psi).  Since Q=16 << D=128 we use the
Woodbury identity: with S = diag(psi), M_k = I + Lam_k^T S^-1 Lam_k = T T^T,
U_k = S^-1 Lam_k T^-T:

  maha_k(x) = d^T S^-1 d - ||U_k^T d||^2,   d = x - mu_k

Expanding in x, the per-sample log responsibility becomes

  log_resps[n,k] = z[n,k] - 0.5*r[n]
  z[n,k]  = const_k + x_n . g_k + || (U_k/sqrt2)^T x_n ||^2
  r[n]    = x_n^T S^-1 x_n

r cancels in the normalized output; it only shifts the log-likelihood.
The device computes, per 128-row tile of X:
  P   = X @ Wh          (Wh = [U_k/sqrt2] stacked, [128, 512])  - PE
  crs = X @ GC + const  (GC = [g_k], [128, 32]; const via rank-1) - PE
  rr  = (X*X) @ (-0.5/s)                                         - PE
  z   = groupsum_16(P^2) + crs ; out = z - lse(z); ll = lse(z)+rr
Host does only the O(K*D*Q) parameter factorization (tiny) and the
shard/unshard.  Sharding: data-parallel over N, 8 ways, no collectives.
"""

import numpy as np

import concourse.bacc as bacc
import concourse.bass as bass
import concourse.mybir as mybir
import concourse.tile as tile
from concourse.bass_utils import run_bass_kernel_spmd

K, D, Q, N = 32, 128, 16, 20000
NCORES = 8
NPAD = 20480          # N padded to 8 * 2560
NLOC = NPAD // NCORES  # 2560 rows per core
PT = 128               # rows per tile (partition dim)
NT = NLOC // PT        # 20 tiles per core
KQ = K * Q             # 512

F32 = mybir.dt.float32
F32R = mybir.dt.float32r
AX = mybir.AxisListType
ALU = mybir.AluOpType
ACT = mybir.ActivationFunctionType

USE_F32R = False


def build_bass():
    """Build the per-core Tile program (same NEFF on all 8 cores)."""
    nc = bacc.Bacc("TRN2", target_bir_lowering=False, debug=False)

    Xs = nc.dram_tensor("Xs", [NLOC, D], F32, kind="ExternalInput")
    Wh = nc.dram_tensor("Wh", [D, KQ], F32, kind="ExternalInput")
    GC = nc.dram_tensor("GC", [D, K], F32, kind="ExternalInput")
    sneg = nc.dram_tensor("sneg", [D, 1], F32, kind="ExternalInput")
    constk = nc.dram_tensor("constk", [1, K], F32, kind="ExternalInput")
    ident = nc.dram_tensor("ident", [PT, PT], F32, kind="ExternalInput")
    out_norm = nc.dram_tensor("out_norm", [NLOC, K], F32, kind="ExternalOutput")
    out_ll = nc.dram_tensor("out_ll", [NLOC, 1], F32, kind="ExternalOutput")

    def mmdt(ap):
        return ap.bitcast(F32R) if USE_F32R else ap

    with tile.TileContext(nc) as tc:
        with (
            tc.tile_pool(name="consts", bufs=1) as cpool,
            tc.tile_pool(name="xin", bufs=3) as xin_pool,
            tc.tile_pool(name="xt", bufs=2) as xt_pool,
            tc.tile_pool(name="x2t", bufs=2) as x2t_pool,
            tc.tile_pool(name="sq", bufs=2) as sq_pool,
            tc.tile_pool(name="small", bufs=3) as spool,
            tc.tile_pool(name="pT", bufs=2, space=bass.MemorySpace.PSUM) as psT_pool,
            tc.tile_pool(name="pP", bufs=2, space=bass.MemorySpace.PSUM) as psP_pool,
            tc.tile_pool(name="pC", bufs=2, space=bass.MemorySpace.PSUM) as psC_pool,
            tc.tile_pool(name="pR", bufs=2, space=bass.MemorySpace.PSUM) as psR_pool,
        ):
            wh_t = cpool.tile([D, KQ], F32)
            nc.sync.dma_start(out=wh_t[:], in_=Wh[:])
            gc_t = cpool.tile([D, K], F32)
            nc.sync.dma_start(out=gc_t[:], in_=GC[:])
            sneg_t = cpool.tile([D, 1], F32)
            nc.sync.dma_start(out=sneg_t[:], in_=sneg[:])
            constk_t = cpool.tile([1, K], F32)
            nc.sync.dma_start(out=constk_t[:], in_=constk[:])
            ident_t = cpool.tile([PT, PT], F32)
            nc.sync.dma_start(out=ident_t[:], in_=ident[:])
            ones_t = cpool.tile([1, PT], F32)
            nc.vector.memset(ones_t[:], 1.0)

            for i in range(NT):
                rows = slice(i * PT, (i + 1) * PT)

                x = xin_pool.tile([PT, D], F32, tag="x")
                nc.sync.dma_start(out=x[:], in_=Xs[rows, :])

                # transpose X tile: psT[d, n] = x[n, d]
                psT = psT_pool.tile([D, PT], F32, tag="psT")
                nc.tensor.transpose(psT[:], x[:], ident_t[:])

                xt = xt_pool.tile([D, PT], F32, tag="xt")
                nc.vector.tensor_copy(xt[:], psT[:])
                x2t = x2t_pool.tile([D, PT], F32, tag="x2t")
                nc.scalar.square(x2t[:], psT[:])

                # P = X @ Wh   -> [n, 512]
                psP = psP_pool.tile([PT, KQ], F32, tag="psP")
                nc.tensor.matmul(psP[:], mmdt(xt[:]), mmdt(wh_t[:]),
                                 start=True, stop=True)
                # crs = X @ GC + 1*const -> [n, 32]
                psC = psC_pool.tile([PT, K], F32, tag="psC")
                nc.tensor.matmul(psC[:], mmdt(xt[:]), mmdt(gc_t[:]),
                                 start=True, stop=False)
                nc.tensor.matmul(psC[:], mmdt(ones_t[:]), mmdt(constk_t[:]),
                                 start=False, stop=True)
                # rr = (X*X) @ sneg -> [n, 1]
                psR = psR_pool.tile([PT, 1], F32, tag="psR")
                nc.tensor.matmul(psR[:], mmdt(x2t[:]), mmdt(sneg_t[:]),
                                 start=True, stop=True)

                # sq = P^2 (ACT, PSUM->SBUF)
                sq = sq_pool.tile([PT, KQ], F32, tag="sq")
                nc.scalar.square(sq[:], psP[:])

                # z0[n,k] = sum_q sq[n, k*16+q] (DVE grouped reduce)
                z0 = spool.tile([PT, K], F32, tag="z0")
                nc.vector.tensor_reduce(
                    z0[:], sq[:].rearrange("p (k q) -> p k q", q=Q),
                    axis=AX.X, op=ALU.add)
                # z = z0 + crs
                z = spool.tile([PT, K], F32, tag="z")
                nc.vector.tensor_add(z[:], z0[:], psC[:])

                # negm = -max_k z
                negm = spool.tile([PT, 1], F32, tag="negm")
                nc.vector.tensor_reduce(negm[:], z[:], axis=AX.X, op=ALU.max,
                                        negate=True)
                # expv = exp(z - m), ssum = sum_k expv
                expv = spool.tile([PT, K], F32, tag="expv")
                ssum = spool.tile([PT, 1], F32, tag="ssum")
                nc.scalar.activation(expv[:], z[:], ACT.Exp,
                                     bias=negm[:, 0:1], scale=1.0,
                                     accum_out=ssum[:])
                # lg = ln(ssum)
                lg = spool.tile([PT, 1], F32, tag="lg")
                nc.scalar.activation(lg[:], ssum[:], ACT.Ln)
                # lse = lg - negm  (ACT: Identity(in*-1 + bias=lg))
                lse = spool.tile([PT, 1], F32, tag="lse")
                nc.scalar.activation(lse[:], negm[:], ACT.Identity,
                                     bias=lg[:, 0:1], scale=-1.0)
                # ll = lse + rr   (ACT: Identity(psR + bias=lse))
                ll = spool.tile([PT, 1], F32, tag="ll")
                nc.scalar.activation(ll[:], psR[:], ACT.Identity,
                                     bias=lse[:, 0:1], scale=1.0)
                # outn = z - lse (per-partition scalar)
                outn = spool.tile([PT, K], F32, tag="outn")
                nc.vector.tensor_scalar(outn[:], z[:], lse[:, 0:1], None,
                                        op0=ALU.subtract)

                nc.sync.dma_start(out=out_norm[rows, :], in_=outn[:])
                nc.sync.dma_start(out=out_ll[rows, :], in_=ll[:])

    nc.compile()
    return nc


def host_precompute(X, log_pi, mu, Lam, log_psi):
    """Tiny O(K*D*Q) parameter factorization, in float64 for accuracy."""
    X = np.asarray(X, np.float32)
    log_pi = np.asarray(log_pi, np.float64)
    mu = np.asarray(mu, np.float64)
    Lam = np.asarray(Lam, np.float64)
    log_psi = np.asarray(log_psi, np.float64)

    s = np.exp(log_psi) + 1e-5 + 1e-4                       # [D]
    sinv = 1.0 / s
    B = Lam * (s ** -0.5)[None, :, None]                    # [K,D,Q]
    M = np.eye(Q)[None] + np.einsum('kdq,kdr->kqr', B, B)   # [K,Q,Q]
    T = np.linalg.cholesky(M)
    logdet = np.sum(np.log(s)) + 2.0 * np.log(
        np.diagonal(T, axis1=1, axis2=2)).sum(1)            # [K]
    Tinv = np.linalg.inv(T)
    U = np.einsum('d,kdq,krq->kdr', sinv, Lam, Tinv)        # [K,D,Q]
    a = sinv[None, :] * mu                                  # [K,D]
    c = np.einsum('kdq,kd->kq', U, mu)                      # [K,Q]
    v = np.einsum('kdq,kq->kd', U, c)                       # [K,D]
    g = a - v                                               # [K,D]
    q1 = np.einsum('kd,kd->k', mu, a)
    q2 = np.einsum('kq,kq->k', c, c)
    const = (log_pi - 0.5 * (D * np.log(2 * np.pi) + logdet)
             - 0.5 * q1 + 0.5 * q2)                         # [K]

    Wh = (U / np.sqrt(2.0)).transpose(0, 2, 1).reshape(KQ, D).T  # [D, KQ]
    return {
        "Wh": np.ascontiguousarray(Wh, dtype=np.float32),
        "GC": np.ascontiguousarray(g.T, dtype=np.float32),
        "sneg": np.ascontiguousarray((-0.5 * sinv)[:, None], dtype=np.float32),
        "constk": np.ascontiguousarray(const[None, :], dtype=np.float32),
        "ident": np.eye(PT, dtype=np.float32),
    }


_NC_CACHE = None


def get_nc():
    global _NC_CACHE
    if _NC_CACHE is None:
        _NC_CACHE = build_bass()
    return _NC_CACHE


def kernel(X, log_pi, mu, Lam, log_psi, _collect=None):
    X = np.asarray(X, np.float32)
    params = host_precompute(X, log_pi, mu, Lam, log_psi)

    Xpad = np.zeros((NPAD, D), dtype=np.float32)
    Xpad[:N] = X
    shards = Xpad.reshape(NCORES, NLOC, D)

    in_maps = [dict(params, Xs=np.ascontiguousarray(shards[c]))
               for c in range(NCORES)]

    nc = get_nc()
    res = run_bass_kernel_spmd(nc, in_maps, list(range(NCORES)),
                               **(_collect or {}))
    if _collect is not None:
        _collect["res"] = res

    norm = np.concatenate([res.results[c]["out_norm"] for c in range(NCORES)],
                          axis=0)[:N]
    ll = np.concatenate([res.results[c]["out_ll"] for c in range(NCORES)],
                        axis=0)[:N, 0]
    return norm, ll


# revision 3
# speedup vs baseline: 1.6571x; 1.6571x over previous
"""MFA e-step (mixture of factor analyzers) on 8 Trainium2 NeuronCores.

Math: the reference computes per-component Gaussian log-likelihoods with
covariance C_k = Lam_k Lam_k^T + diag(psi).  Since Q=16 << D=128 we use the
Woodbury identity: with S = diag(psi), M_k = I + Lam_k^T S^-1 Lam_k = T T^T,
U_k = S^-1 Lam_k T^-T:

  maha_k(x) = d^T S^-1 d - ||U_k^T d||^2,   d = x - mu_k

Expanding in x, the per-sample log responsibility becomes

  log_resps[n,k] = z[n,k] - 0.5*r[n]
  z[n,k]  = const_k + x_n . g_k + || (U_k/sqrt2)^T x_n ||^2
  r[n]    = x_n^T S^-1 x_n

r cancels in the normalized output; it only shifts the log-likelihood.
The device computes, per 128-row tile of X (X is fed pre-transposed, D on
partitions):
  P   = X @ Wh          (Wh = [U_k/sqrt2] stacked, [128, 512])  - PE, fp32r
  crs = X @ GC + const  (GC = [g_k], [128, 32]; const via rank-1) - PE
  rr  = (X*X) @ (-0.5/s)                                         - PE
  z   = groupsum_16(P^2) + crs                                   - ACT+DVE
then one batched logsumexp over all 20 tiles (single exp / single ln, so
the ACT engine loads its function tables at most twice).
Host does only the O(K*D*Q) parameter factorization (tiny) and the
shard/unshard.  Sharding: data-parallel over N, 8 ways, no collectives.
"""

import numpy as np

import concourse.bacc as bacc
import concourse.bass as bass
import concourse.mybir as mybir
import concourse.tile as tile
from concourse.bass_utils import run_bass_kernel_spmd

K, D, Q, N = 32, 128, 16, 20000
NCORES = 8
NPAD = 20480          # N padded to 8 * 2560
NLOC = NPAD // NCORES  # 2560 rows per core
PT = 128               # rows per tile (partition dim)
NT = NLOC // PT        # 20 tiles per core
KQ = K * Q             # 512

F32 = mybir.dt.float32
F32R = mybir.dt.float32r
AX = mybir.AxisListType
ALU = mybir.AluOpType
ACTF = mybir.ActivationFunctionType

USE_F32R = True


def build_bass():
    """Build the per-core Tile program (same NEFF on all 8 cores)."""
    nc = bacc.Bacc("TRN2", target_bir_lowering=False, debug=False)

    # X shard arrives pre-transposed: [D, NLOC], so tiles DMA straight into
    # the matmul operand layout (D on partitions) with no on-chip transpose.
    XsT = nc.dram_tensor("XsT", [D, NLOC], F32, kind="ExternalInput")
    Wh = nc.dram_tensor("Wh", [D, KQ], F32, kind="ExternalInput")
    GC = nc.dram_tensor("GC", [D, K], F32, kind="ExternalInput")
    sneg = nc.dram_tensor("sneg", [D, 1], F32, kind="ExternalInput")
    constk = nc.dram_tensor("constk", [1, K], F32, kind="ExternalInput")
    out_norm = nc.dram_tensor("out_norm", [NLOC, K], F32, kind="ExternalOutput")
    out_ll = nc.dram_tensor("out_ll", [NLOC, 1], F32, kind="ExternalOutput")

    with tile.TileContext(nc) as tc:
        with (
            tc.tile_pool(name="consts", bufs=1) as cpool,
            tc.tile_pool(name="xbig", bufs=1) as xbig,
            tc.tile_pool(name="x2t", bufs=2) as x2t_pool,
            tc.tile_pool(name="sq", bufs=3) as sq_pool,
            tc.tile_pool(name="acc", bufs=1) as accp,
            tc.tile_pool(name="small", bufs=2) as spool,
            tc.tile_pool(name="pP", bufs=2, space=bass.MemorySpace.PSUM) as psP_pool,
            tc.tile_pool(name="pC", bufs=2, space=bass.MemorySpace.PSUM) as psC_pool,
            tc.tile_pool(name="pR", bufs=2, space=bass.MemorySpace.PSUM) as psR_pool,
        ):
            gc_t = cpool.tile([D, K], F32)
            nc.sync.dma_start(out=gc_t[:], in_=GC[:])
            sneg_t = cpool.tile([D, 1], F32)
            nc.sync.dma_start(out=sneg_t[:], in_=sneg[:])
            constk_t = cpool.tile([1, K], F32)
            nc.sync.dma_start(out=constk_t[:], in_=constk[:])
            ones_t = cpool.tile([1, PT], F32)
            nc.vector.memset(ones_t[:], 1.0)

            if USE_F32R:
                # fp32r operands must come from a rounding producer; DMA the
                # fp32 originals and round once through the DVE.
                wh_stage = cpool.tile([D, KQ], F32)
                nc.sync.dma_start(out=wh_stage[:], in_=Wh[:])
                wh_t = cpool.tile([D, KQ], F32R)
                nc.vector.tensor_copy(wh_t[:], wh_stage[:])
            else:
                wh_t = cpool.tile([D, KQ], F32)
                nc.sync.dma_start(out=wh_t[:], in_=Wh[:])

            # whole X shard in SBUF, transposed layout [D, NLOC]
            xt_all = xbig.tile([D, NLOC], F32)
            half = NLOC // 2
            nc.sync.dma_start(out=xt_all[:, :half], in_=XsT[:, :half])
            nc.sync.dma_start(out=xt_all[:, half:], in_=XsT[:, half:])
            if USE_F32R:
                xtr_all = xbig.tile([D, NLOC], F32R)
                nc.vector.tensor_copy(xtr_all[:, :half], xt_all[:, :half])
                nc.vector.tensor_copy(xtr_all[:, half:], xt_all[:, half:])
            else:
                xtr_all = xt_all

            # accumulators across all tiles
            z_all = accp.tile([PT, NT, K], F32)     # z per (row, tile, k)
            rr_all = accp.tile([PT, NT], F32)       # -0.5 r per (row, tile)

            for i in range(NT):
                cols = slice(i * PT, (i + 1) * PT)
                xt = xt_all[:, cols]
                xtr = xtr_all[:, cols]

                # P = X @ Wh   -> [n, 512]
                psP = psP_pool.tile([PT, KQ], F32, tag="psP")
                nc.tensor.matmul(psP[:], xtr, wh_t[:], start=True, stop=True)
                # crs = X @ GC + 1*const -> [n, 32]
                psC = psC_pool.tile([PT, K], F32, tag="psC")
                nc.tensor.matmul(psC[:], xt, gc_t[:], start=True, stop=False)
                nc.tensor.matmul(psC[:], ones_t[:], constk_t[:],
                                 start=False, stop=True)
                # rr = (X*X) @ sneg -> [n, 1]
                x2t = x2t_pool.tile([D, PT], F32, tag="x2t")
                nc.scalar.square(x2t[:], xt)
                psR = psR_pool.tile([PT, 1], F32, tag="psR")
                nc.tensor.matmul(psR[:], x2t[:], sneg_t[:], start=True, stop=True)

                # sq = P^2 (ACT, PSUM->SBUF)
                sq = sq_pool.tile([PT, KQ], F32, tag="sq")
                nc.scalar.square(sq[:], psP[:])

                # z0[n,k] = sum_q sq[n, k*16+q] (DVE grouped reduce)
                z0 = spool.tile([PT, K], F32, tag="z0")
                nc.vector.tensor_reduce(
                    z0[:], sq[:].rearrange("p (k q) -> p k q", q=Q),
                    axis=AX.X, op=ALU.add)
                # z = z0 + crs -> slice of z_all
                nc.vector.tensor_add(z_all[:, i, :], z0[:], psC[:])
                # rr slice
                nc.vector.tensor_copy(rr_all[:, i:i + 1], psR[:])

            # ---- batched logsumexp over all tiles ----
            zf = z_all[:].rearrange("p t k -> p (t k)")
            negm = spool.tile([PT, 1], F32, tag="negm")
            nc.vector.tensor_reduce(negm[:], zf, axis=AX.X, op=ALU.max,
                                    negate=True)
            ev = accp.tile([PT, NT, K], F32)
            nc.scalar.activation(ev[:].rearrange("p t k -> p (t k)"), zf,
                                 ACTF.Exp, bias=negm[:, 0:1], scale=1.0)
            ssum = spool.tile([PT, NT], F32, tag="ssum")
            nc.vector.tensor_reduce(ssum[:], ev[:], axis=AX.X, op=ALU.add)
            lg = spool.tile([PT, NT], F32, tag="lg")
            nc.scalar.activation(lg[:], ssum[:], ACTF.Ln)
            # lse[p,t] = lg[p,t] + m[p] = lg - negm
            lse = spool.tile([PT, NT], F32, tag="lse")
            nc.vector.tensor_scalar(lse[:], lg[:], negm[:, 0:1], None,
                                    op0=ALU.subtract)
            # ll = lse + rr
            ll = spool.tile([PT, NT], F32, tag="ll")
            nc.vector.tensor_add(ll[:], lse[:], rr_all[:])
            # outn = z - lse (broadcast along k)
            outn = accp.tile([PT, NT, K], F32)
            lse_b = lse[:].unsqueeze(2).broadcast_to([PT, NT, K])
            nc.vector.tensor_sub(outn[:], z_all[:], lse_b)

            # ---- batched outputs ----
            # out_norm[(t*128+p), k] = outn[p, t, k]
            on_view = out_norm.ap().rearrange("(t p) k -> p t k", p=PT)
            nc.sync.dma_start(out=on_view, in_=outn[:])
            oll_view = out_ll.ap().rearrange("(t p) one -> p (t one)", p=PT)
            nc.sync.dma_start(out=oll_view, in_=ll[:])

    nc.compile()
    return nc


def host_precompute(X, log_pi, mu, Lam, log_psi):
    """Tiny O(K*D*Q) parameter factorization, in float64 for accuracy."""
    log_pi = np.asarray(log_pi, np.float64)
    mu = np.asarray(mu, np.float64)
    Lam = np.asarray(Lam, np.float64)
    log_psi = np.asarray(log_psi, np.float64)

    s = np.exp(log_psi) + 1e-5 + 1e-4                       # [D]
    sinv = 1.0 / s
    B = Lam * (s ** -0.5)[None, :, None]                    # [K,D,Q]
    M = np.eye(Q)[None] + np.einsum('kdq,kdr->kqr', B, B)   # [K,Q,Q]
    T = np.linalg.cholesky(M)
    logdet = np.sum(np.log(s)) + 2.0 * np.log(
        np.diagonal(T, axis1=1, axis2=2)).sum(1)            # [K]
    Tinv = np.linalg.inv(T)
    U = np.einsum('d,kdq,krq->kdr', sinv, Lam, Tinv)        # [K,D,Q]
    a = sinv[None, :] * mu                                  # [K,D]
    c = np.einsum('kdq,kd->kq', U, mu)                      # [K,Q]
    v = np.einsum('kdq,kq->kd', U, c)                       # [K,D]
    g = a - v                                               # [K,D]
    q1 = np.einsum('kd,kd->k', mu, a)
    q2 = np.einsum('kq,kq->k', c, c)
    const = (log_pi - 0.5 * (D * np.log(2 * np.pi) + logdet)
             - 0.5 * q1 + 0.5 * q2)                         # [K]

    Wh = (U / np.sqrt(2.0)).transpose(0, 2, 1).reshape(KQ, D).T  # [D, KQ]
    return {
        "Wh": np.ascontiguousarray(Wh, dtype=np.float32),
        "GC": np.ascontiguousarray(g.T, dtype=np.float32),
        "sneg": np.ascontiguousarray((-0.5 * sinv)[:, None], dtype=np.float32),
        "constk": np.ascontiguousarray(const[None, :], dtype=np.float32),
    }


_NC_CACHE = None


def get_nc():
    global _NC_CACHE
    if _NC_CACHE is None:
        _NC_CACHE = build_bass()
    return _NC_CACHE


def kernel(X, log_pi, mu, Lam, log_psi, _collect=None):
    X = np.asarray(X, np.float32)
    params = host_precompute(X, log_pi, mu, Lam, log_psi)

    Xpad = np.zeros((NPAD, D), dtype=np.float32)
    Xpad[:N] = X
    # per-core transposed shards [D, NLOC]
    shards = Xpad.reshape(NCORES, NLOC, D)

    in_maps = [dict(params, XsT=np.ascontiguousarray(shards[c].T))
               for c in range(NCORES)]

    nc = get_nc()
    res = run_bass_kernel_spmd(nc, in_maps, list(range(NCORES)),
                               **(_collect or {}))
    if _collect is not None:
        _collect["res"] = res

    # device row order within a core is (t*128 + p); it matches the shard's
    # natural row order, so plain concatenation restores global order.
    norm = np.concatenate([res.results[c]["out_norm"] for c in range(NCORES)],
                          axis=0)[:N]
    ll = np.concatenate([res.results[c]["out_ll"] for c in range(NCORES)],
                        axis=0)[:N, 0]
    return norm, ll


# revision 11
# speedup vs baseline: 2.1887x; 1.3208x over previous
"""MFA e-step (mixture of factor analyzers) on 8 Trainium2 NeuronCores.

Math: the reference computes per-component Gaussian log-likelihoods with
covariance C_k = Lam_k Lam_k^T + diag(psi).  Since Q=16 << D=128 we use the
Woodbury identity: with S = diag(psi), M_k = I + Lam_k^T S^-1 Lam_k = T T^T,
U_k = S^-1 Lam_k T^-T:

  maha_k(x) = d^T S^-1 d - ||U_k^T d||^2,   d = x - mu_k

Expanding in x, the per-sample log responsibility becomes

  log_resps[n,k] = z[n,k] - 0.5*r[n]
  z[n,k]  = const_k + x_n . g_k + || (U_k/sqrt2)^T x_n ||^2
  r[n]    = x_n^T S^-1 x_n

r cancels in the normalized output; it only shifts the log-likelihood.
The device computes, per 128-row tile of X (X is fed pre-transposed, D on
partitions):
  P   = X @ Wh          (Wh = [U_k/sqrt2] stacked, [128, 512])  - PE, fp32r
  crs = X @ GC + const  (GC = [g_k], [128, 32]; const via rank-1) - PE
  rr  = (X*X) @ (-0.5/s)                                         - PE
  z   = groupsum_16(P^2) + crs                                   - ACT+DVE
then one batched logsumexp over all 20 tiles (single exp / single ln, so
the ACT engine loads its function tables at most twice).
Host does only the O(K*D*Q) parameter factorization (tiny) and the
shard/unshard.  Sharding: data-parallel over N, 8 ways, no collectives.
"""

import numpy as np

import concourse.bacc as bacc
import concourse.bass as bass
import concourse.mybir as mybir
import concourse.tile as tile
from concourse.bass_utils import run_bass_kernel_spmd

K, D, Q, N = 32, 128, 16, 20000
NCORES = 8
NPAD = 20480          # N padded to 8 * 2560
NLOC = NPAD // NCORES  # 2560 rows per core
PT = 128               # rows per tile (partition dim)
NT = NLOC // PT        # 20 tiles per core
KQ = K * Q             # 512

F32 = mybir.dt.float32
F32R = mybir.dt.float32r
AX = mybir.AxisListType
ALU = mybir.AluOpType
ACTF = mybir.ActivationFunctionType

USE_F32R = True       # main P matmul in fp32r
AUX_F32R = True       # crs / rr matmuls in fp32r


def build_bass():
    """Build the per-core Tile program (same NEFF on all 8 cores)."""
    nc = bacc.Bacc("TRN2", target_bir_lowering=False, debug=False)

    # X shard arrives pre-transposed: [D, NLOC], so tiles DMA straight into
    # the matmul operand layout (D on partitions) with no on-chip transpose.
    XsT = nc.dram_tensor("XsT", [D, NLOC], F32, kind="ExternalInput")
    Wh = nc.dram_tensor("Wh", [D, KQ], F32, kind="ExternalInput")
    GC = nc.dram_tensor("GC", [D, K], F32, kind="ExternalInput")
    sneg = nc.dram_tensor("sneg", [D, 2], F32, kind="ExternalInput")
    constb = nc.dram_tensor("constb", [PT, K], F32, kind="ExternalInput")
    out_norm = nc.dram_tensor("out_norm", [NLOC, K], F32, kind="ExternalOutput")
    out_ll = nc.dram_tensor("out_ll", [NLOC, 1], F32, kind="ExternalOutput")

    with tile.TileContext(nc) as tc:
        with (
            tc.tile_pool(name="consts", bufs=1) as cpool,
            tc.tile_pool(name="xbig", bufs=1) as xbig,
            tc.tile_pool(name="x2t", bufs=2) as x2t_pool,
            tc.tile_pool(name="sq", bufs=3) as sq_pool,
            tc.tile_pool(name="acc", bufs=1) as accp,
            tc.tile_pool(name="small", bufs=2) as spool,
            tc.tile_pool(name="pP", bufs=2, space=bass.MemorySpace.PSUM) as psP_pool,
            tc.tile_pool(name="pC", bufs=2, space=bass.MemorySpace.PSUM) as psC_pool,
            tc.tile_pool(name="pR", bufs=2, space=bass.MemorySpace.PSUM) as psR_pool,
        ):
            def load_const(name, dram, shape, rdt):
                t = cpool.tile(shape, F32, tag=name)
                nc.sync.dma_start(out=t[:], in_=dram[:])
                if rdt == F32:
                    return t
                tr = cpool.tile(shape, F32R, tag=name + "_r")
                nc.vector.tensor_copy(tr[:], t[:])
                return tr

            auxdt = F32R if AUX_F32R else F32
            maindt = F32R if USE_F32R else F32
            gc_t = load_const("gc", GC, [D, K], auxdt)
            # fp32r matmuls need an even output free size; sneg is [D,2]
            # host-side with a zero second column.
            sneg_t = load_const("sneg", sneg, [D, 2], auxdt)
            wh_t = load_const("wh", Wh, [D, KQ], maindt)
            constb_t = cpool.tile([PT, K], F32, tag="constb")
            nc.sync.dma_start(out=constb_t[:], in_=constb[:])

            # whole X shard in SBUF, transposed layout [D, NLOC]
            xt_all = xbig.tile([D, NLOC], F32)
            half = NLOC // 2
            nc.sync.dma_start(out=xt_all[:, :half], in_=XsT[:, :half])
            nc.sync.dma_start(out=xt_all[:, half:], in_=XsT[:, half:])
            if USE_F32R or AUX_F32R:
                xtr_all = xbig.tile([D, NLOC], F32R)
                nc.vector.tensor_copy(xtr_all[:, :half], xt_all[:, :half])
                nc.vector.tensor_copy(xtr_all[:, half:], xt_all[:, half:])
            else:
                xtr_all = xt_all

            xm_all = xtr_all if USE_F32R else xt_all
            xa_all = xtr_all if AUX_F32R else xt_all

            # accumulators across all tiles
            z_all = accp.tile([PT, NT, K], F32)     # z per (row, tile, k)
            rr_all = accp.tile([PT, NT], F32)       # -0.5 r per (row, tile)

            for i in range(NT):
                cols = slice(i * PT, (i + 1) * PT)

                # P = X @ Wh   -> [n, 512]
                psP = psP_pool.tile([PT, KQ], F32, tag="psP")
                nc.tensor.matmul(psP[:], xm_all[:, cols], wh_t[:],
                                 start=True, stop=True)
                # crs = X @ GC -> [n, 32]
                psC = psC_pool.tile([PT, K], F32, tag="psC")
                nc.tensor.matmul(psC[:], xa_all[:, cols], gc_t[:],
                                 start=True, stop=True)
                # rr = (X*X) @ sneg -> [n, 1]
                x2t = x2t_pool.tile([D, PT], F32R if AUX_F32R else F32,
                                    tag="x2t")
                nc.scalar.square(x2t[:], xt_all[:, cols])
                psR = psR_pool.tile([PT, 2], F32, tag="psR")
                nc.tensor.matmul(psR[:], x2t[:], sneg_t[:], start=True, stop=True)

                # sq = P^2 (ACT, PSUM->SBUF)
                sq = sq_pool.tile([PT, KQ], F32, tag="sq")
                nc.scalar.square(sq[:], psP[:])

                # z0[n,k] = sum_q sq[n, k*16+q] (DVE grouped reduce)
                nc.vector.tensor_reduce(
                    z_all[:, i, :], sq[:].rearrange("p (k q) -> p k q", q=Q),
                    axis=AX.X, op=ALU.add)
                # z += crs (DVE reads PSUM), z += const (GpSimd, SBUF only)
                nc.vector.tensor_add(z_all[:, i, :], z_all[:, i, :], psC[:])
                nc.gpsimd.tensor_add(z_all[:, i, :], z_all[:, i, :],
                                     constb_t[:])
                # rr slice
                nc.vector.tensor_copy(rr_all[:, i:i + 1], psR[:, 0:1])

            # ---- batched logsumexp over all tiles ----
            zf = z_all[:].rearrange("p t k -> p (t k)")
            negm = spool.tile([PT, 1], F32, tag="negm")
            nc.vector.tensor_reduce(negm[:], zf, axis=AX.X, op=ALU.max,
                                    negate=True)
            ev = accp.tile([PT, NT, K], F32)
            nc.scalar.activation(ev[:].rearrange("p t k -> p (t k)"), zf,
                                 ACTF.Exp, bias=negm[:, 0:1], scale=1.0)
            ssum = spool.tile([PT, NT], F32, tag="ssum")
            nc.vector.tensor_reduce(ssum[:], ev[:], axis=AX.X, op=ALU.add)
            lg = spool.tile([PT, NT], F32, tag="lg")
            nc.scalar.activation(lg[:], ssum[:], ACTF.Ln)
            # lse[p,t] = lg[p,t] + m[p] = lg - negm
            lse = spool.tile([PT, NT], F32, tag="lse")
            nc.vector.tensor_scalar(lse[:], lg[:], negm[:, 0:1], None,
                                    op0=ALU.subtract)
            # ll = lse + rr
            ll = spool.tile([PT, NT], F32, tag="ll")
            nc.vector.tensor_add(ll[:], lse[:], rr_all[:])
            # outn = z - lse (broadcast along k)
            outn = accp.tile([PT, NT, K], F32)
            lse_b = lse[:].unsqueeze(2).broadcast_to([PT, NT, K])
            nc.vector.tensor_sub(outn[:], z_all[:], lse_b)

            # ---- batched outputs ----
            # out_norm[(t*128+p), k] = outn[p, t, k]
            on_view = out_norm.ap().rearrange("(t p) k -> p t k", p=PT)
            nc.sync.dma_start(out=on_view, in_=outn[:])
            oll_view = out_ll.ap().rearrange("(t p) one -> p (t one)", p=PT)
            nc.sync.dma_start(out=oll_view, in_=ll[:])

    nc.compile()
    return nc


def host_precompute(X, log_pi, mu, Lam, log_psi):
    """Tiny O(K*D*Q) parameter factorization, in float64 for accuracy."""
    log_pi = np.asarray(log_pi, np.float64)
    mu = np.asarray(mu, np.float64)
    Lam = np.asarray(Lam, np.float64)
    log_psi = np.asarray(log_psi, np.float64)

    s = np.exp(log_psi) + 1e-5 + 1e-4                       # [D]
    sinv = 1.0 / s
    B = Lam * (s ** -0.5)[None, :, None]                    # [K,D,Q]
    M = np.eye(Q)[None] + np.einsum('kdq,kdr->kqr', B, B)   # [K,Q,Q]
    T = np.linalg.cholesky(M)
    logdet = np.sum(np.log(s)) + 2.0 * np.log(
        np.diagonal(T, axis1=1, axis2=2)).sum(1)            # [K]
    Tinv = np.linalg.inv(T)
    U = np.einsum('d,kdq,krq->kdr', sinv, Lam, Tinv)        # [K,D,Q]
    a = sinv[None, :] * mu                                  # [K,D]
    c = np.einsum('kdq,kd->kq', U, mu)                      # [K,Q]
    v = np.einsum('kdq,kq->kd', U, c)                       # [K,D]
    g = a - v                                               # [K,D]
    q1 = np.einsum('kd,kd->k', mu, a)
    q2 = np.einsum('kq,kq->k', c, c)
    const = (log_pi - 0.5 * (D * np.log(2 * np.pi) + logdet)
             - 0.5 * q1 + 0.5 * q2)                         # [K]

    Wh = (U / np.sqrt(2.0)).transpose(0, 2, 1).reshape(KQ, D).T  # [D, KQ]
    return {
        "Wh": np.ascontiguousarray(Wh, dtype=np.float32),
        "GC": np.ascontiguousarray(g.T, dtype=np.float32),
        "sneg": np.ascontiguousarray(
            np.stack([-0.5 * sinv, np.zeros(D)], axis=1), dtype=np.float32),
        "constb": np.ascontiguousarray(
            np.broadcast_to(const[None, :], (PT, K)), dtype=np.float32),
    }


_NC_CACHE = None


def get_nc():
    global _NC_CACHE
    if _NC_CACHE is None:
        _NC_CACHE = build_bass()
    return _NC_CACHE


def kernel(X, log_pi, mu, Lam, log_psi, _collect=None):
    X = np.asarray(X, np.float32)
    params = host_precompute(X, log_pi, mu, Lam, log_psi)

    Xpad = np.zeros((NPAD, D), dtype=np.float32)
    Xpad[:N] = X
    # per-core transposed shards [D, NLOC]
    shards = Xpad.reshape(NCORES, NLOC, D)

    in_maps = [dict(params, XsT=np.ascontiguousarray(shards[c].T))
               for c in range(NCORES)]

    nc = get_nc()
    res = run_bass_kernel_spmd(nc, in_maps, list(range(NCORES)),
                               **(_collect or {}))
    if _collect is not None:
        _collect["res"] = res

    # device row order within a core is (t*128 + p); it matches the shard's
    # natural row order, so plain concatenation restores global order.
    norm = np.concatenate([res.results[c]["out_norm"] for c in range(NCORES)],
                          axis=0)[:N]
    ll = np.concatenate([res.results[c]["out_ll"] for c in range(NCORES)],
                        axis=0)[:N, 0]
    return norm, ll


# revision 17
# speedup vs baseline: 2.3304x; 1.0648x over previous
"""MFA e-step (mixture of factor analyzers) on 8 Trainium2 NeuronCores.

Math: the reference computes per-component Gaussian log-likelihoods with
covariance C_k = Lam_k Lam_k^T + diag(psi).  Since Q=16 << D=128 we use the
Woodbury identity: with S = diag(psi), M_k = I + Lam_k^T S^-1 Lam_k = T T^T,
U_k = S^-1 Lam_k T^-T:

  maha_k(x) = d^T S^-1 d - ||U_k^T d||^2,   d = x - mu_k

Expanding in x, the per-sample log responsibility becomes

  log_resps[n,k] = z[n,k] - 0.5*r[n]
  z[n,k]  = const_k + x_n . g_k + || (U_k/sqrt2)^T x_n ||^2
  r[n]    = x_n^T S^-1 x_n

r cancels in the normalized output; it only shifts the log-likelihood.
The device computes, per 128-row tile of X (X is fed pre-transposed, D on
partitions):
  P   = X @ Wh          (Wh = [U_k/sqrt2] stacked, [128, 512])  - PE, fp32r
  crs = X @ GC + const  (GC = [g_k], [128, 32]; const via rank-1) - PE
  rr  = (X*X) @ (-0.5/s)                                         - PE
  z   = groupsum_16(P^2) + crs                                   - ACT+DVE
then one batched logsumexp over all 20 tiles (single exp / single ln, so
the ACT engine loads its function tables at most twice).
Host does only the O(K*D*Q) parameter factorization (tiny) and the
shard/unshard.  Sharding: data-parallel over N, 8 ways, no collectives.
"""

import numpy as np

import concourse.bacc as bacc
import concourse.bass as bass
import concourse.mybir as mybir
import concourse.tile as tile
from concourse.bass_utils import run_bass_kernel_spmd

K, D, Q, N = 32, 128, 16, 20000
NCORES = 8
NPAD = 20480          # N padded to 8 * 2560
NLOC = NPAD // NCORES  # 2560 rows per core
PT = 128               # rows per tile (partition dim)
NT = NLOC // PT        # 20 tiles per core
KQ = K * Q             # 512

F32 = mybir.dt.float32
F32R = mybir.dt.float32r
AX = mybir.AxisListType
ALU = mybir.AluOpType
ACTF = mybir.ActivationFunctionType

USE_F32R = True       # main P matmul in fp32r
AUX_F32R = True       # crs / rr matmuls in fp32r
POOL_REDUCE = False   # grouped sum-of-squares via pool_avg instead of reduce
PH2_CHUNKS = 2        # logsumexp phases overlapping the main loop
PSP_BUFS = 3
SQ_BUFS = 4


def build_bass():
    """Build the per-core Tile program (same NEFF on all 8 cores)."""
    nc = bacc.Bacc("TRN2", target_bir_lowering=False, debug=False)

    # X shard arrives pre-transposed: [D, NLOC], so tiles DMA straight into
    # the matmul operand layout (D on partitions) with no on-chip transpose.
    XsT = nc.dram_tensor("XsT", [D, NLOC], F32, kind="ExternalInput")
    Wh = nc.dram_tensor("Wh", [D, KQ], F32, kind="ExternalInput")
    GC = nc.dram_tensor("GC", [D, K], F32, kind="ExternalInput")
    sneg = nc.dram_tensor("sneg", [D, 2], F32, kind="ExternalInput")
    constb = nc.dram_tensor("constb", [PT, K], F32, kind="ExternalInput")
    out_norm = nc.dram_tensor("out_norm", [NLOC, K], F32, kind="ExternalOutput")
    out_ll = nc.dram_tensor("out_ll", [NLOC, 1], F32, kind="ExternalOutput")

    with tile.TileContext(nc) as tc:
        with (
            tc.tile_pool(name="consts", bufs=1) as cpool,
            tc.tile_pool(name="xbig", bufs=1) as xbig,
            tc.tile_pool(name="sq", bufs=SQ_BUFS) as sq_pool,
            tc.tile_pool(name="acc", bufs=1) as accp,
            tc.tile_pool(name="small", bufs=2) as spool,
            tc.tile_pool(name="pP", bufs=PSP_BUFS,
                         space=bass.MemorySpace.PSUM) as psP_pool,
            tc.tile_pool(name="pC", bufs=2, space=bass.MemorySpace.PSUM) as psC_pool,
            tc.tile_pool(name="pR", bufs=2, space=bass.MemorySpace.PSUM) as psR_pool,
        ):
            def load_const(name, dram, shape, rdt):
                t = cpool.tile(shape, F32, tag=name)
                nc.sync.dma_start(out=t[:], in_=dram[:])
                if rdt == F32:
                    return t
                tr = cpool.tile(shape, F32R, tag=name + "_r")
                nc.vector.tensor_copy(tr[:], t[:])
                return tr

            auxdt = F32R if AUX_F32R else F32
            maindt = F32R if USE_F32R else F32
            gc_t = load_const("gc", GC, [D, K], auxdt)
            # fp32r matmuls need an even output free size; sneg is [D,2]
            # host-side with a zero second column.
            sneg_t = load_const("sneg", sneg, [D, 2], auxdt)
            wh_t = load_const("wh", Wh, [D, KQ], maindt)
            constb_t = cpool.tile([PT, K], F32, tag="constb")
            nc.sync.dma_start(out=constb_t[:], in_=constb[:])

            # whole X shard in SBUF, transposed layout [D, NLOC]
            xt_all = xbig.tile([D, NLOC], F32)
            half = NLOC // 2
            nc.sync.dma_start(out=xt_all[:, :half], in_=XsT[:, :half])
            nc.sync.dma_start(out=xt_all[:, half:], in_=XsT[:, half:])
            if USE_F32R or AUX_F32R:
                xtr_all = xbig.tile([D, NLOC], F32R)
                nc.vector.tensor_copy(xtr_all[:, :half], xt_all[:, :half])
                nc.vector.tensor_copy(xtr_all[:, half:], xt_all[:, half:])
            else:
                xtr_all = xt_all

            xm_all = xtr_all if USE_F32R else xt_all
            xa_all = xtr_all if AUX_F32R else xt_all

            # x^2, whole shard, batched (2 chunks)
            x2_all = xbig.tile([D, NLOC], F32R if AUX_F32R else F32)
            nc.scalar.square(x2_all[:, :half], xt_all[:, :half])
            nc.scalar.square(x2_all[:, half:], xt_all[:, half:])

            # accumulators across all tiles
            z_all = accp.tile([PT, NT, K], F32)     # z per (row, tile, k)
            rr_all = accp.tile([PT, NT], F32)       # -0.5 r per (row, tile)
            ev = accp.tile([PT, NT, K], F32)
            outn = accp.tile([PT, NT, K], F32)

            CH = NT // PH2_CHUNKS

            def phase2(c):
                """Batched logsumexp for tiles [c*CH, (c+1)*CH)."""
                ts = slice(c * CH, (c + 1) * CH)
                zf = z_all[:, ts, :].rearrange("p t k -> p (t k)")
                negm = spool.tile([PT, 1], F32, tag="negm")
                nc.vector.tensor_reduce(negm[:], zf, axis=AX.X, op=ALU.max,
                                        negate=True)
                nc.scalar.activation(
                    ev[:, ts, :].rearrange("p t k -> p (t k)"), zf,
                    ACTF.Exp, bias=negm[:, 0:1], scale=1.0)
                ssum = spool.tile([PT, CH], F32, tag="ssum")
                nc.vector.tensor_reduce(ssum[:], ev[:, ts, :], axis=AX.X,
                                        op=ALU.add)
                lg = spool.tile([PT, CH], F32, tag="lg")
                nc.scalar.activation(lg[:], ssum[:], ACTF.Ln)
                # lse[p,t] = lg[p,t] + m[p] = lg - negm
                lse = spool.tile([PT, CH], F32, tag="lse")
                nc.vector.tensor_scalar(lse[:], lg[:], negm[:, 0:1], None,
                                        op0=ALU.subtract)
                # ll = lse + rr
                ll = spool.tile([PT, CH], F32, tag="ll")
                nc.vector.tensor_add(ll[:], lse[:], rr_all[:, ts])
                # outn = z - lse (broadcast along k)
                lse_b = lse[:].unsqueeze(2).broadcast_to([PT, CH, K])
                nc.vector.tensor_sub(outn[:, ts, :], z_all[:, ts, :], lse_b)

                # out_norm[(t*128+p), k] = outn[p, t, k]
                on_view = out_norm.ap().rearrange("(t p) k -> p t k", p=PT)
                nc.sync.dma_start(out=on_view[:, ts, :], in_=outn[:, ts, :])
                oll_view = out_ll.ap().rearrange("(t p) one -> p (t one)",
                                                 p=PT)
                nc.sync.dma_start(out=oll_view[:, ts], in_=ll[:])

            for i in range(NT):
                cols = slice(i * PT, (i + 1) * PT)

                # P = X @ Wh   -> [n, 512]
                psP = psP_pool.tile([PT, KQ], F32, tag="psP")
                nc.tensor.matmul(psP[:], xm_all[:, cols], wh_t[:],
                                 start=True, stop=True)
                # crs = X @ GC -> [n, 32]
                psC = psC_pool.tile([PT, K], F32, tag="psC")
                nc.tensor.matmul(psC[:], xa_all[:, cols], gc_t[:],
                                 start=True, stop=True)
                # rr = (X*X) @ sneg -> [n, 1]
                psR = psR_pool.tile([PT, 2], F32, tag="psR")
                nc.tensor.matmul(psR[:], x2_all[:, cols], sneg_t[:],
                                 start=True, stop=True)

                # sq = P^2 (ACT, PSUM->SBUF)
                sq = sq_pool.tile([PT, KQ], F32, tag="sq")
                nc.scalar.square(sq[:], psP[:])

                # z0[n,k] = sum_q sq[n, k*16+q] (DVE grouped reduce)
                sqg = sq[:].rearrange("p (k q) -> p k q", q=Q)
                if POOL_REDUCE:
                    sqg5 = sq[:].rearrange("p (a b k q) -> p a b k q",
                                           a=1, b=1, q=Q)
                    nc.vector.pool(z_all[:, i, :], sqg5,
                                   func=mybir.PoolFunctionType.avg)
                else:
                    nc.vector.tensor_reduce(z_all[:, i, :], sqg,
                                            axis=AX.X, op=ALU.add)
                # z += crs (DVE reads PSUM), z += const (GpSimd, SBUF only)
                nc.vector.tensor_add(z_all[:, i, :], z_all[:, i, :], psC[:])
                nc.gpsimd.tensor_add(z_all[:, i, :], z_all[:, i, :],
                                     constb_t[:])
                # rr slice
                nc.vector.tensor_copy(rr_all[:, i:i + 1], psR[:, 0:1])

                if (i + 1) % CH == 0:
                    phase2(i // CH)

    nc.compile()
    return nc


def host_precompute(X, log_pi, mu, Lam, log_psi):
    """Tiny O(K*D*Q) parameter factorization, in float64 for accuracy."""
    log_pi = np.asarray(log_pi, np.float64)
    mu = np.asarray(mu, np.float64)
    Lam = np.asarray(Lam, np.float64)
    log_psi = np.asarray(log_psi, np.float64)

    s = np.exp(log_psi) + 1e-5 + 1e-4                       # [D]
    sinv = 1.0 / s
    B = Lam * (s ** -0.5)[None, :, None]                    # [K,D,Q]
    M = np.eye(Q)[None] + np.einsum('kdq,kdr->kqr', B, B)   # [K,Q,Q]
    T = np.linalg.cholesky(M)
    logdet = np.sum(np.log(s)) + 2.0 * np.log(
        np.diagonal(T, axis1=1, axis2=2)).sum(1)            # [K]
    Tinv = np.linalg.inv(T)
    U = np.einsum('d,kdq,krq->kdr', sinv, Lam, Tinv)        # [K,D,Q]
    a = sinv[None, :] * mu                                  # [K,D]
    c = np.einsum('kdq,kd->kq', U, mu)                      # [K,Q]
    v = np.einsum('kdq,kq->kd', U, c)                       # [K,D]
    g = a - v                                               # [K,D]
    q1 = np.einsum('kd,kd->k', mu, a)
    q2 = np.einsum('kq,kq->k', c, c)
    const = (log_pi - 0.5 * (D * np.log(2 * np.pi) + logdet)
             - 0.5 * q1 + 0.5 * q2)                         # [K]

    # scale so that the device's grouped reduce (plain sum, or avg-pool which
    # divides by Q) yields exactly 0.5 * ||U^T x||^2
    wscale = np.sqrt(Q / 2.0) if POOL_REDUCE else np.sqrt(0.5)
    Wh = (U * wscale).transpose(0, 2, 1).reshape(KQ, D).T  # [D, KQ]
    return {
        "Wh": np.ascontiguousarray(Wh, dtype=np.float32),
        "GC": np.ascontiguousarray(g.T, dtype=np.float32),
        "sneg": np.ascontiguousarray(
            np.stack([-0.5 * sinv, np.zeros(D)], axis=1), dtype=np.float32),
        "constb": np.ascontiguousarray(
            np.broadcast_to(const[None, :], (PT, K)), dtype=np.float32),
    }


_NC_CACHE = None


def get_nc():
    global _NC_CACHE
    if _NC_CACHE is None:
        _NC_CACHE = build_bass()
    return _NC_CACHE


def kernel(X, log_pi, mu, Lam, log_psi, _collect=None):
    X = np.asarray(X, np.float32)
    params = host_precompute(X, log_pi, mu, Lam, log_psi)

    Xpad = np.zeros((NPAD, D), dtype=np.float32)
    Xpad[:N] = X
    # per-core transposed shards [D, NLOC]
    shards = Xpad.reshape(NCORES, NLOC, D)

    in_maps = [dict(params, XsT=np.ascontiguousarray(shards[c].T))
               for c in range(NCORES)]

    nc = get_nc()
    res = run_bass_kernel_spmd(nc, in_maps, list(range(NCORES)),
                               **(_collect or {}))
    if _collect is not None:
        _collect["res"] = res

    # device row order within a core is (t*128 + p); it matches the shard's
    # natural row order, so plain concatenation restores global order.
    norm = np.concatenate([res.results[c]["out_norm"] for c in range(NCORES)],
                          axis=0)[:N]
    ll = np.concatenate([res.results[c]["out_ll"] for c in range(NCORES)],
                        axis=0)[:N, 0]
    return norm, ll
